# revision 19
# baseline (speedup 1.0000x reference)
"""Trainium2 Bass kernel for nn_DEQLatentSpaceOpt (DDIM trajectory DEQ iteration).

Computation (reference):
    xT = x[0:1]
    repeat 3x:  et = conv3x3(xt[:T]) + temb[t][:,:,None,None]
                xt_next = alpha_ratio*xT + epc * cumsum(et_coeff * et, axis=0)
                xt = concat([xT, xt_next])

Sharding: T=1000 trajectory rows split as 125 rows per core across 8 cores.
Per core, images are packed 3-per-partition-group: partition p = 3g + c
(g in 0..41 group, c channel), image local index l = 3g + j (slot j in 0..2).
The 3x3 conv runs on TensorE as 9 accumulating block-diagonal matmuls over a
row-padded (stride-66) bf16 image layout; shifted taps are plain AP offsets.
The cumsum along the trajectory + all per-timestep coefficients + the
cross-core carry + the alpha_ratio*xT term are folded into host-precomputed
triangular matmul weights (PE does all the math; fp32 PSUM accumulation).
Cross-core prefix: one 8-rank AllGather of per-core totals per iteration.
"""

import numpy as np
import ml_dtypes

import concourse.bass as bass
import concourse.bacc as bacc
import concourse.mybir as mybir
import concourse.tile as tile
from concourse.bass_utils import run_bass_kernel_spmd
from concourse.bass_interp import get_hw_module

BF16 = mybir.dt.bfloat16
F32 = mybir.dt.float32

N_CORES = 8
T = 1000
C = 3
HW = 4096  # 64*64
TLOC = T // N_CORES  # 125 rows per core
G = 42  # partition groups; partition p = 3g + c, 126 used of 128
S = 3  # image slots per partition (42*3 = 126 slots >= 125 images)
N_ITER = 3

# padded image layout per partition: row stride 66 (1 left pad + 64 px + 1
# right pad), one 66-wide gap row between images, one lead gap row.
ROWS = S * 65 + 1  # 196
RW = 66
TAPS = [(dy, dx) for dy in (-1, 0, 1) for dx in (-1, 0, 1)]
CHUNK_ROWS = 8  # conv matmul chunk: 8 image rows x 64 px = 512 cols
NCH = 64 // CHUNK_ROWS  # 8 chunks per image slot
PS_GRP = 2  # psum tile holds 2 chunks (1024 f32 = 2 banks)

_compiled = None


def _build_module():
    nc = bacc.Bacc(
        "TRN2", target_bir_lowering=False, debug=False, num_devices=N_CORES
    )

    # I/O
    x_arr = nc.dram_tensor("x_arr", [128, S, HW], F32, kind="ExternalInput").ap()
    xt_bf = nc.dram_tensor("xt_bf", [C, HW], BF16, kind="ExternalInput").ap()
    w9 = nc.dram_tensor("w9", [9, 128, 128], BF16, kind="ExternalInput").ap()
    triw = nc.dram_tensor("triw", [9, 128, 128], BF16, kind="ExternalInput").ap()
    cxw = nc.dram_tensor("cxw", [S, 27, 128], BF16, kind="ExternalInput").ap()
    totw = nc.dram_tensor("totw", [S, 128, C], BF16, kind="ExternalInput").ap()
    biasw = nc.dram_tensor("biasw", [128, S], F32, kind="ExternalInput").ap()
    out_arr = nc.dram_tensor("out_arr", [128, S, HW], F32, kind="ExternalOutput").ap()

    TRI_IDX = {(j, l): 3 * j + l for j in range(S) for l in range(S)}

    with tile.TileContext(nc) as tc:
        with (
            tc.tile_pool(name="persist", bufs=1) as pp,
            tc.tile_pool(name="work", bufs=2) as wp,
            tc.tile_pool(name="pconv", bufs=2, space="PSUM") as pconv,
            tc.tile_pool(name="pmisc", bufs=2, space="PSUM") as pmisc,
            tc.tile_pool(name="dram", bufs=2, space="DRAM") as dp,
        ):
            # persistent tiles
            convin = pp.tile([128, ROWS, RW], BF16, tag="convin")
            stag = pp.tile([128, S, HW], F32, tag="stag")
            e = pp.tile([128, S, HW], BF16, tag="e")
            rhs_cx = pp.tile([27, HW], BF16, tag="rhs_cx")
            agin_s = pp.tile([C, HW], BF16, tag="agin_s")
            w9s = pp.tile([128, 9, 128], BF16, tag="w9s")
            tris = pp.tile([128, 9, 128], BF16, tag="tris")
            cxs = pp.tile([27, S, 128], BF16, tag="cxs")
            tots = pp.tile([128, S, C], BF16, tag="tots")
            biass = pp.tile([128, S], F32, tag="biass")

            # zero the padded conv input once (pads must stay zero)
            nc.gpsimd.memset(convin[:], 0.0)

            # load coefficients (w9/tris/cxs have leading dim as free axis on
            # 128 partitions; DMA per plane keeps partition dim = 128)
            for i in range(9):
                nc.sync.dma_start(w9s[:, i], w9[i])
            for i in range(9):
                nc.sync.dma_start(tris[:, i], triw[i])
            for j in range(S):
                nc.sync.dma_start(cxs[:, j], cxw[j])
                nc.sync.dma_start(tots[:, j], totw[j])
            nc.sync.dma_start(biass[:], biasw[:])
            nc.sync.dma_start(rhs_cx[24:27, :], xt_bf[:])

            # load x (staging) and convert to padded bf16, per slot
            for j in range(S):
                nc.sync.dma_start(stag[:, j], x_arr[:, j])
                r0 = 1 + 65 * j
                nc.vector.tensor_copy(
                    convin[:, r0 : r0 + 64, 1:65],
                    stag[:, j].rearrange("p (a b) -> p a b", a=64),
                )

            for it in range(N_ITER):
                last = it == N_ITER - 1

                # ---- conv: 9 shifted block-diag matmuls per chunk ----
                for j in range(S):
                    r0 = 1 + 65 * j
                    for cg in range(NCH // PS_GRP):  # chunk groups of 2
                        pt = pconv.tile([128, PS_GRP * 512], F32, tag="pconv")
                        for ci in range(PS_GRP):
                            ch = cg * PS_GRP + ci
                            rr = r0 + ch * CHUNK_ROWS
                            for ti, (dy, dx) in enumerate(TAPS):
                                nc.tensor.matmul(
                                    pt[:, ci * 512 : (ci + 1) * 512],
                                    w9s[:, ti],
                                    convin[
                                        :,
                                        rr + dy : rr + CHUNK_ROWS + dy,
                                        1 + dx : 65 + dx,
                                    ],
                                    start=(ti == 0),
                                    stop=(ti == 8),
                                )
                        # evac: ACT copy + per-partition temb bias -> e (bf16)
                        c0 = cg * PS_GRP * 512
                        nc.scalar.activation(
                            e[:, j, c0 : c0 + PS_GRP * 512],
                            pt[:],
                            mybir.ActivationFunctionType.Identity,
                            bias=biass[:, j : j + 1],
                        )

                # ---- per-core totals -> AllGather ----
                for ch in range(NCH):
                    ptt = pmisc.tile([C, 512], F32, tag="pmisc")
                    for l in range(S):
                        nc.tensor.matmul(
                            ptt[:],
                            tots[:, l],
                            e[:, l, ch * 512 : (ch + 1) * 512],
                            start=(l == 0),
                            stop=(l == S - 1),
                        )
                    nc.vector.tensor_copy(agin_s[:, ch * 512 : (ch + 1) * 512], ptt[:])
                ag_in = dp.tile([C, HW], BF16, tag="ag_in")
                ag_out = dp.tile([N_CORES * C, HW], BF16, tag="ag_out")
                nc.sync.dma_start(ag_in[:], agin_s[:])
                nc.gpsimd.collective_compute(
                    "AllGather",
                    mybir.AluOpType.bypass,
                    replica_groups=[list(range(N_CORES))],
                    ins=[ag_in.opt()],
                    outs=[ag_out.opt()],
                )
                nc.sync.dma_start(rhs_cx[0:24, :], ag_out[:])

                # ---- combine: carry/xT + triangular cumsum matmuls ----
                # j=2 weights are pre-shifted by +3 output partitions and
                # carry the boundary row in columns 0..2 (see _build_inputs),
                # so every writeback is partition-0-aligned.
                for j in range(S):
                    for cg in range(NCH // PS_GRP):
                        pc = pmisc.tile([128, PS_GRP * 512], F32, tag="pmisc")
                        for ci in range(PS_GRP):
                            c0 = (cg * PS_GRP + ci) * 512
                            sl = slice(ci * 512, (ci + 1) * 512)
                            nc.tensor.matmul(
                                pc[:, sl],
                                cxs[:, j],
                                rhs_cx[:, c0 : c0 + 512],
                                start=True,
                                stop=False,
                            )
                            for l in range(S):
                                nc.tensor.matmul(
                                    pc[:, sl],
                                    tris[:, TRI_IDX[(j, l)]],
                                    e[:, l, c0 : c0 + 512],
                                    start=False,
                                    stop=(l == S - 1),
                                )
                        # writeback
                        rows = PS_GRP * CHUNK_ROWS
                        c0 = cg * PS_GRP * 512
                        if last:
                            nc.scalar.activation(
                                stag[:, j, c0 : c0 + PS_GRP * 512],
                                pc[:],
                                mybir.ActivationFunctionType.Copy,
                            )
                        else:
                            # image l=3g+j -> next xt image l+1 (slot j+1, or
                            # slot 0 via the pre-shifted j=2 weights)
                            jd = j + 1 if j < S - 1 else 0
                            rr = 1 + 65 * jd + cg * rows
                            nc.scalar.activation(
                                convin[0:126, rr : rr + rows, 1:65],
                                pc[0:126].rearrange("p (a b) -> p a b", b=64),
                                mybir.ActivationFunctionType.Copy,
                            )

            # final output
            nc.sync.dma_start(out_arr[:], stag[:])

    nc.compile()
    nc.m = get_hw_module(nc.m)
    return nc


def _build_inputs(x, alpha_ratio, et_coeff, et_prevsum_coeff, conv_w, temb, t):
    """Host-side coefficient precompute; returns per-core in_maps."""
    ar = np.asarray(alpha_ratio, np.float64).reshape(T)
    etc = np.asarray(et_coeff, np.float64).reshape(T)
    epc = np.asarray(et_prevsum_coeff, np.float64).reshape(T)
    temb = np.asarray(temb, np.float32)
    t = np.asarray(t).astype(np.int64)
    conv_w = np.asarray(conv_w, np.float32)
    x = np.asarray(x, np.float32)
    tembsel = temb[t]  # [T, C] bias per trajectory row

    bf = ml_dtypes.bfloat16

    # shared: conv tap weights, block-diagonal [3g+ci, 3g+co]
    w9 = np.zeros((9, 128, 128), np.float32)
    for ti, (dy, dx) in enumerate(TAPS):
        blk = conv_w[:, :, dy + 1, dx + 1].T  # [ci, co]
        for g in range(G):
            w9[ti, 3 * g : 3 * g + 3, 3 * g : 3 * g + 3] = blk
    w9 = w9.astype(bf)

    xt_b = x[0].reshape(C, HW).astype(bf)

    gs = np.arange(G)
    in_maps = []
    for k in range(N_CORES):
        o = k * TLOC

        def idx(g, j):
            return o + 3 * g + j

        def valid(g, j):
            return 3 * g + j <= TLOC - 1

        vmask = np.array([[valid(g, j) for j in range(S)] for g in range(G)])

        # j=2 combine outputs are shifted +3 partitions (next xt slot (g+1,0))
        # and columns 0..2 hold the boundary row xt_next[o-1].
        def ocol(g, j):
            return 3 * (g + 1) if j == S - 1 else 3 * g

        tri = np.zeros((9, 128, 128), np.float32)
        for j in range(S):
            for l in range(S):
                ti = 3 * j + l
                for g in range(G):
                    if not vmask[g, j]:
                        continue
                    glim = g + 1 if l <= j else g  # 3g'+l <= 3g+j
                    if glim == 0:
                        continue
                    gp = gs[:glim]
                    vv = vmask[gp, l]
                    w = etc[idx(gp, l)] * epc[idx(g, j)] * vv
                    oc = ocol(g, j)
                    if oc + 3 > 128:
                        continue
                    for c in range(C):
                        tri[ti, 3 * gp + c, oc + c] = w

        cx = np.zeros((S, 27, 128), np.float32)
        for j in range(S):
            for g in range(G):
                if not vmask[g, j]:
                    continue
                oc = ocol(g, j)
                if oc + 3 > 128:
                    continue
                for c in range(C):
                    cx[j, 3 * np.arange(k) + c, oc + c] = epc[idx(g, j)]
                    cx[j, 24 + c, oc + c] = ar[idx(g, j)]
        # boundary row -> j=2 columns 0..2
        epc_b = epc[o - 1] if k > 0 else 0.0
        ar_b = ar[o - 1] if k > 0 else 1.0
        for c in range(C):
            cx[S - 1, 3 * np.arange(k) + c, c] = epc_b
            cx[S - 1, 24 + c, c] = ar_b

        tot = np.zeros((S, 128, C), np.float32)
        for l in range(S):
            for g in range(G):
                if vmask[g, l]:
                    for c in range(C):
                        tot[l, 3 * g + c, c] = etc[idx(g, l)]

        bias = np.zeros((128, S), np.float32)
        for j in range(S):
            for g in range(G):
                if vmask[g, j]:
                    bias[3 * g : 3 * g + 3, j] = tembsel[idx(g, j)]

        xa = np.zeros((128, S, HW), np.float32)
        for j in range(S):
            rows = o + 3 * gs + j  # x row index for slot (g, j); <= 1000
            xa[3 * gs[:, None] + np.arange(C), j] = x[rows].reshape(G, C, HW)

        in_maps.append(
            {
                "x_arr": xa,
                "xt_bf": xt_b,
                "w9": w9,
                "triw": tri.astype(bf),
                "cxw": cx.astype(bf),
                "totw": tot.astype(bf),
                "biasw": bias,
            }
        )
    return in_maps


def kernel(x, t, alpha_ratio, et_coeff, et_prevsum_coeff, conv_w, temb):
    global _compiled
    if _compiled is None:
        _compiled = _build_module()
    nc = _compiled

    in_maps = _build_inputs(x, alpha_ratio, et_coeff, et_prevsum_coeff, conv_w, temb, t)
    res = run_bass_kernel_spmd(nc, in_maps, core_ids=list(range(N_CORES)))

    x = np.asarray(x, np.float32)
    y = np.empty((T + 1, C, 64, 64), np.float32)
    y[0] = x[0]
    gs = np.arange(G)
    for k in range(N_CORES):
        o = k * TLOC
        oa = res.results[k]["out_arr"]  # [128, S, HW]
        for j in range(S):
            gv = gs[3 * gs + j <= TLOC - 1]
            if j == S - 1:
                # shifted layout: partition group g+1 holds image 3g+2
                gp = gv + 1
                rows = o + 3 * gp  # = o + (3g+2) + 1
                y[rows] = oa[(3 * gp[:, None] + np.arange(C)), j].reshape(
                    len(gp), C, 64, 64
                )
            else:
                rows = o + 3 * gv + j + 1
                y[rows] = oa[(3 * gv[:, None] + np.arange(C)), j].reshape(
                    len(gv), C, 64, 64
                )
    return y


# revision 25
# speedup vs baseline: 1.0817x; 1.0817x over previous
"""Trainium2 Bass kernel for nn_DEQLatentSpaceOpt (DDIM trajectory DEQ iteration).

Computation (reference):
    xT = x[0:1]
    repeat 3x:  et = conv3x3(xt[:T]) + temb[t][:,:,None,None]
                xt_next = alpha_ratio*xT + epc * cumsum(et_coeff * et, axis=0)
                xt = concat([xT, xt_next])

Sharding: T=1000 trajectory rows split as 125 rows per core across 8 cores.
Per core, images are packed 3-per-partition-group: partition p = 3g + c
(g in 0..41 group, c channel), image local index l = 3g + j (slot j in 0..2).
The 3x3 conv runs on TensorE as 9 accumulating block-diagonal matmuls over a
row-padded (stride-66) bf16 image layout; shifted taps are plain AP offsets.
The cumsum along the trajectory + all per-timestep coefficients + the
cross-core carry + the alpha_ratio*xT term are folded into host-precomputed
triangular matmul weights (PE does all the math; fp32 PSUM accumulation).
Cross-core prefix: one 8-rank AllGather of per-core totals per iteration.
"""

import numpy as np
import ml_dtypes

import jax
import concourse.bass as bass
import concourse.bacc as bacc
import concourse.mybir as mybir
import concourse.tile as tile
from concourse.bass_utils import run_bass_kernel_spmd
from concourse.bass_interp import get_hw_module
from concourse import bass2jax

BF16 = mybir.dt.bfloat16
F32 = mybir.dt.float32

N_CORES = 8
T = 1000
C = 3
HW = 4096  # 64*64
TLOC = T // N_CORES  # 125 rows per core
G = 42  # partition groups; partition p = 3g + c, 126 used of 128
S = 3  # image slots per partition (42*3 = 126 slots >= 125 images)
N_ITER = 3

# padded image layout per partition: row stride 66 (1 left pad + 64 px + 1
# right pad), one 66-wide gap row between images, one lead gap row.
ROWS = S * 65 + 1  # 196
RW = 66
TAPS = [(dy, dx) for dy in (-1, 0, 1) for dx in (-1, 0, 1)]
CHUNK_ROWS = 8  # conv matmul chunk: 8 image rows x 64 px = 512 cols
NCH = 64 // CHUNK_ROWS  # 8 chunks per image slot
PS_GRP = 2  # psum tile holds 2 chunks (1024 f32 = 2 banks)

_compiled = None


def _build_module():
    nc = bacc.Bacc(
        "TRN2", target_bir_lowering=False, debug=False, num_devices=N_CORES
    )

    # I/O
    x_arr = nc.dram_tensor("x_arr", [128, S, HW], F32, kind="ExternalInput").ap()
    xt_bf = nc.dram_tensor("xt_bf", [C, HW], BF16, kind="ExternalInput").ap()
    w9 = nc.dram_tensor("w9", [9, 128, 128], BF16, kind="ExternalInput").ap()
    triw = nc.dram_tensor("triw", [9, 128, 128], BF16, kind="ExternalInput").ap()
    cxw = nc.dram_tensor("cxw", [S, 27, 128], BF16, kind="ExternalInput").ap()
    totw = nc.dram_tensor("totw", [S, 128, C], BF16, kind="ExternalInput").ap()
    biasw = nc.dram_tensor("biasw", [128, S], F32, kind="ExternalInput").ap()
    out_arr = nc.dram_tensor("out_arr", [128, S, HW], F32, kind="ExternalOutput").ap()

    TRI_IDX = {(j, l): 3 * j + l for j in range(S) for l in range(S)}

    with tile.TileContext(nc) as tc:
        with (
            tc.tile_pool(name="persist", bufs=1) as pp,
            tc.tile_pool(name="work", bufs=2) as wp,
            tc.tile_pool(name="pconv", bufs=2, space="PSUM") as pconv,
            tc.tile_pool(name="pmisc", bufs=2, space="PSUM") as pmisc,
            tc.tile_pool(name="dram", bufs=2, space="DRAM") as dp,
        ):
            # persistent tiles
            convin = pp.tile([128, ROWS, RW], BF16, tag="convin")
            stag = pp.tile([128, S, HW], F32, tag="stag")
            e = pp.tile([128, S, HW], BF16, tag="e")
            rhs_cx = pp.tile([27, HW], BF16, tag="rhs_cx")
            agin_s = pp.tile([C, HW], BF16, tag="agin_s")
            w9s = pp.tile([128, 9, 128], BF16, tag="w9s")
            tris = pp.tile([128, 9, 128], BF16, tag="tris")
            cxs = pp.tile([27, S, 128], BF16, tag="cxs")
            tots = pp.tile([128, S, C], BF16, tag="tots")
            biass = pp.tile([128, S], F32, tag="biass")

            # zero the padded conv input once (pads must stay zero)
            nc.gpsimd.memset(convin[:], 0.0)

            # load coefficients (w9/tris/cxs have leading dim as free axis on
            # 128 partitions; DMA per plane keeps partition dim = 128)
            for i in range(9):
                nc.sync.dma_start(w9s[:, i], w9[i])
            for i in range(9):
                nc.sync.dma_start(tris[:, i], triw[i])
            for j in range(S):
                nc.sync.dma_start(cxs[:, j], cxw[j])
                nc.sync.dma_start(tots[:, j], totw[j])
            nc.sync.dma_start(biass[:], biasw[:])
            nc.sync.dma_start(rhs_cx[24:27, :], xt_bf[:])

            # load x (staging) and convert to padded bf16, per slot
            for j in range(S):
                nc.sync.dma_start(stag[:, j], x_arr[:, j])
                r0 = 1 + 65 * j
                nc.vector.tensor_copy(
                    convin[:, r0 : r0 + 64, 1:65],
                    stag[:, j].rearrange("p (a b) -> p a b", a=64),
                )

            for it in range(N_ITER):
                last = it == N_ITER - 1

                # ---- conv: 9 shifted block-diag matmuls per chunk ----
                for j in range(S):
                    r0 = 1 + 65 * j
                    for cg in range(NCH // PS_GRP):  # chunk groups of 2
                        pt = pconv.tile([128, PS_GRP * 512], F32, tag="pconv")
                        for ci in range(PS_GRP):
                            ch = cg * PS_GRP + ci
                            rr = r0 + ch * CHUNK_ROWS
                            for ti, (dy, dx) in enumerate(TAPS):
                                nc.tensor.matmul(
                                    pt[:, ci * 512 : (ci + 1) * 512],
                                    w9s[:, ti],
                                    convin[
                                        :,
                                        rr + dy : rr + CHUNK_ROWS + dy,
                                        1 + dx : 65 + dx,
                                    ],
                                    start=(ti == 0),
                                    stop=(ti == 8),
                                )
                        # evac: ACT copy + per-partition temb bias -> e (bf16)
                        c0 = cg * PS_GRP * 512
                        nc.scalar.activation(
                            e[:, j, c0 : c0 + PS_GRP * 512],
                            pt[:],
                            mybir.ActivationFunctionType.Identity,
                            bias=biass[:, j : j + 1],
                        )

                # ---- per-core totals -> AllGather ----
                for ch in range(NCH):
                    ptt = pmisc.tile([C, 512], F32, tag="pmisc")
                    for l in range(S):
                        nc.tensor.matmul(
                            ptt[:],
                            tots[:, l],
                            e[:, l, ch * 512 : (ch + 1) * 512],
                            start=(l == 0),
                            stop=(l == S - 1),
                        )
                    nc.vector.tensor_copy(agin_s[:, ch * 512 : (ch + 1) * 512], ptt[:])
                ag_in = dp.tile([C, HW], BF16, tag="ag_in")
                ag_out = dp.tile([N_CORES * C, HW], BF16, tag="ag_out")
                nc.sync.dma_start(ag_in[:], agin_s[:])
                nc.gpsimd.collective_compute(
                    "AllGather",
                    mybir.AluOpType.bypass,
                    replica_groups=[list(range(N_CORES))],
                    ins=[ag_in.opt()],
                    outs=[ag_out.opt()],
                )
                nc.sync.dma_start(rhs_cx[0:24, :], ag_out[:])

                # ---- combine: carry/xT + triangular cumsum matmuls ----
                # j=2 weights are pre-shifted by +3 output partitions and
                # carry the boundary row in columns 0..2 (see _build_inputs),
                # so every writeback is partition-0-aligned.
                for j in range(S):
                    for cg in range(NCH // PS_GRP):
                        pc = pmisc.tile([128, PS_GRP * 512], F32, tag="pmisc")
                        for ci in range(PS_GRP):
                            c0 = (cg * PS_GRP + ci) * 512
                            sl = slice(ci * 512, (ci + 1) * 512)
                            nc.tensor.matmul(
                                pc[:, sl],
                                cxs[:, j],
                                rhs_cx[:, c0 : c0 + 512],
                                start=True,
                                stop=False,
                            )
                            for l in range(S):
                                nc.tensor.matmul(
                                    pc[:, sl],
                                    tris[:, TRI_IDX[(j, l)]],
                                    e[:, l, c0 : c0 + 512],
                                    start=False,
                                    stop=(l == S - 1),
                                )
                        # writeback
                        rows = PS_GRP * CHUNK_ROWS
                        c0 = cg * PS_GRP * 512
                        if last:
                            nc.scalar.activation(
                                stag[:, j, c0 : c0 + PS_GRP * 512],
                                pc[:],
                                mybir.ActivationFunctionType.Copy,
                            )
                        else:
                            # image l=3g+j -> next xt image l+1 (slot j+1, or
                            # slot 0 via the pre-shifted j=2 weights)
                            jd = j + 1 if j < S - 1 else 0
                            rr = 1 + 65 * jd + cg * rows
                            nc.scalar.activation(
                                convin[0:126, rr : rr + rows, 1:65],
                                pc[0:126].rearrange("p (a b) -> p a b", b=64),
                                mybir.ActivationFunctionType.Copy,
                            )

            # final output
            nc.sync.dma_start(out_arr[:], stag[:])

    nc.compile()
    nc.m = get_hw_module(nc.m)
    return nc


def _build_inputs(x, alpha_ratio, et_coeff, et_prevsum_coeff, conv_w, temb, t):
    """Host-side coefficient precompute; returns per-core in_maps."""
    ar = np.asarray(alpha_ratio, np.float64).reshape(T)
    etc = np.asarray(et_coeff, np.float64).reshape(T)
    epc = np.asarray(et_prevsum_coeff, np.float64).reshape(T)
    temb = np.asarray(temb, np.float32)
    t = np.asarray(t).astype(np.int64)
    conv_w = np.asarray(conv_w, np.float32)
    x = np.asarray(x, np.float32)
    tembsel = temb[t]  # [T, C] bias per trajectory row

    bf = ml_dtypes.bfloat16

    # shared: conv tap weights, block-diagonal [3g+ci, 3g+co]
    w9 = np.zeros((9, 128, 128), np.float32)
    for ti, (dy, dx) in enumerate(TAPS):
        blk = conv_w[:, :, dy + 1, dx + 1].T  # [ci, co]
        for g in range(G):
            w9[ti, 3 * g : 3 * g + 3, 3 * g : 3 * g + 3] = blk
    w9 = w9.astype(bf)

    xt_b = x[0].reshape(C, HW).astype(bf)

    gs = np.arange(G)
    in_maps = []
    for k in range(N_CORES):
        o = k * TLOC

        def idx(g, j):
            return o + 3 * g + j

        def valid(g, j):
            return 3 * g + j <= TLOC - 1

        vmask = np.array([[valid(g, j) for j in range(S)] for g in range(G)])

        # j=2 combine outputs are shifted +3 partitions (next xt slot (g+1,0))
        # and columns 0..2 hold the boundary row xt_next[o-1].
        def ocol(g, j):
            return 3 * (g + 1) if j == S - 1 else 3 * g

        tri = np.zeros((9, 128, 128), np.float32)
        for j in range(S):
            for l in range(S):
                ti = 3 * j + l
                for g in range(G):
                    if not vmask[g, j]:
                        continue
                    glim = g + 1 if l <= j else g  # 3g'+l <= 3g+j
                    if glim == 0:
                        continue
                    gp = gs[:glim]
                    vv = vmask[gp, l]
                    w = etc[idx(gp, l)] * epc[idx(g, j)] * vv
                    oc = ocol(g, j)
                    if oc + 3 > 128:
                        continue
                    for c in range(C):
                        tri[ti, 3 * gp + c, oc + c] = w

        cx = np.zeros((S, 27, 128), np.float32)
        for j in range(S):
            for g in range(G):
                if not vmask[g, j]:
                    continue
                oc = ocol(g, j)
                if oc + 3 > 128:
                    continue
                for c in range(C):
                    cx[j, 3 * np.arange(k) + c, oc + c] = epc[idx(g, j)]
                    cx[j, 24 + c, oc + c] = ar[idx(g, j)]
        # boundary row -> j=2 columns 0..2
        epc_b = epc[o - 1] if k > 0 else 0.0
        ar_b = ar[o - 1] if k > 0 else 1.0
        for c in range(C):
            cx[S - 1, 3 * np.arange(k) + c, c] = epc_b
            cx[S - 1, 24 + c, c] = ar_b

        tot = np.zeros((S, 128, C), np.float32)
        for l in range(S):
            for g in range(G):
                if vmask[g, l]:
                    for c in range(C):
                        tot[l, 3 * g + c, c] = etc[idx(g, l)]

        bias = np.zeros((128, S), np.float32)
        for j in range(S):
            for g in range(G):
                if vmask[g, j]:
                    bias[3 * g : 3 * g + 3, j] = tembsel[idx(g, j)]

        xa = np.zeros((128, S, HW), np.float32)
        for j in range(S):
            rows = o + 3 * gs + j  # x row index for slot (g, j); <= 1000
            xa[3 * gs[:, None] + np.arange(C), j] = x[rows].reshape(G, C, HW)

        in_maps.append(
            {
                "x_arr": xa,
                "xt_bf": xt_b,
                "w9": w9,
                "triw": tri.astype(bf),
                "cxw": cx.astype(bf),
                "totw": tot.astype(bf),
                "biasw": bias,
            }
        )
    return in_maps


class _Runner:
    """Compile once, keep the jitted sharded executable for reuse."""

    def __init__(self):
        from jax.sharding import Mesh, PartitionSpec
        from jax.experimental.shard_map import shard_map

        self.nc = _build_module()
        nc = self.nc
        bass2jax.install_neuronx_cc_hook()

        part_name = (
            nc.partition_id_tensor.name if nc.partition_id_tensor else None
        )
        in_names, out_names, out_avals, zero_shapes = [], [], [], []
        for alloc in nc.m.functions[0].allocations:
            if not isinstance(alloc, mybir.MemoryLocationSet):
                continue
            name = alloc.memorylocations[0].name
            if alloc.kind == "ExternalInput":
                if name != part_name:
                    in_names.append(name)
            elif alloc.kind == "ExternalOutput":
                out_names.append(name)
                shape = tuple(alloc.tensor_shape)
                dtype = mybir.dt.np(alloc.dtype)
                out_avals.append(jax.core.ShapedArray(shape, dtype))
                zero_shapes.append((shape, dtype))
        n_params = len(in_names)
        n_outs = len(out_names)
        all_names = in_names + out_names
        if part_name is not None:
            all_names = all_names + [part_name]
        self.in_names = in_names
        self.out_names = out_names
        self.n_params = n_params
        self.zero_shapes = zero_shapes

        def _body(*args):
            operands = list(args)
            if part_name is not None:
                operands.append(bass2jax.partition_id_tensor())
            outs = bass2jax._bass_exec_p.bind(
                *operands,
                out_avals=tuple(out_avals),
                in_names=tuple(all_names),
                out_names=tuple(out_names),
                lowering_input_output_aliases=(),
                sim_require_finite=True,
                sim_require_nnan=True,
                nc=nc,
            )
            return tuple(outs)

        devices = jax.devices()[:N_CORES]
        mesh = Mesh(np.asarray(devices), ("core",))
        in_specs = (PartitionSpec("core"),) * (n_params + n_outs)
        out_specs = (PartitionSpec("core"),) * n_outs
        self.fn = jax.jit(
            shard_map(
                _body, mesh=mesh, in_specs=in_specs, out_specs=out_specs,
                check_rep=False,
            ),
            donate_argnums=tuple(range(n_params, n_params + n_outs)),
            keep_unused=True,
        )

    def __call__(self, in_maps):
        concat_in = [
            np.concatenate([np.asarray(m[name]) for m in in_maps], axis=0)
            for name in self.in_names
        ]
        zeros = [
            np.zeros((N_CORES * s[0], *s[1:]), d) for s, d in self.zero_shapes
        ]
        outs = self.fn(*concat_in, *zeros)
        return [
            {
                name: np.asarray(outs[i]).reshape(N_CORES, -1, *outs[i].shape[1:])[c]
                for i, name in enumerate(self.out_names)
            }
            for c in range(N_CORES)
        ]


def kernel(x, t, alpha_ratio, et_coeff, et_prevsum_coeff, conv_w, temb):
    global _compiled
    if _compiled is None:
        _compiled = _Runner()

    in_maps = _build_inputs(x, alpha_ratio, et_coeff, et_prevsum_coeff, conv_w, temb, t)
    results = _compiled(in_maps)

    x = np.asarray(x, np.float32)
    y = np.empty((T + 1, C, 64, 64), np.float32)
    y[0] = x[0]
    gs = np.arange(G)
    for k in range(N_CORES):
        o = k * TLOC
        oa = results[k]["out_arr"]  # [128, S, HW]
        for j in range(S):
            gv = gs[3 * gs + j <= TLOC - 1]
            if j == S - 1:
                # shifted layout: partition group g+1 holds image 3g+2
                gp = gv + 1
                rows = o + 3 * gp  # = o + (3g+2) + 1
                y[rows] = oa[(3 * gp[:, None] + np.arange(C)), j].reshape(
                    len(gp), C, 64, 64
                )
            else:
                rows = o + 3 * gv + j + 1
                y[rows] = oa[(3 * gv[:, None] + np.arange(C)), j].reshape(
                    len(gv), C, 64, 64
                )
    return y
